# revision 7
# baseline (speedup 1.0000x reference)
"""Trainium2 Bass kernel for nn_CrossAttentionModule (4-scale cross-attention + semantic MLP).

Sharding: 8 cores; core c handles sample b=c//2, query-half h=c%2.
 - scales 0-2: query dim split across the core pair; K/V computed redundantly per pair.
 - scale 3 (tiny): computed fully on both cores of the pair (host takes even core's).
 - semantic path: pooling/sg-conv per sample, BatchNorm batch stats via one tiny
   AllReduce over all 8 cores (each pair contributes its sample twice -> divide by 2B*HW),
   fc1 full per core, fc2 rows split across the pair.
Attention per core: S^T = K^T(chunk) Q (m on partitions), exp on ACT, denominator via an
appended ones-column in V^T (AV matmul also yields row sums), normalization folded into the
PSUM->SBUF eviction scale, PE transpose of av^T, then output projection + residual.
"""
import os
import sys

sys.path.insert(0, '/opt/trn_rl_repo')

import numpy as np

import concourse.bass as bass
import concourse.tile as tile
from concourse import bacc, mybir
from concourse.bass_utils import run_bass_kernel_spmd
from concourse.masks import make_identity

TD = 256
NCORES = 8
F32 = mybir.dt.float32
MDT = mybir.dt.float32r  # fast PE path; 1 cyc/row for free-dim >= 256

SCALES = [
    dict(cin=64, Nk=4096, Nq=2048, yg=8, yd=8),
    dict(cin=128, Nk=1024, Nq=512, yg=8, yd=4),
    dict(cin=256, Nk=256, Nq=128, yg=8, yd=2),
    dict(cin=512, Nk=64, Nq=64, yg=8, yd=1),
]

LAST = {}


def _ceil(a, b):
    return (a + b - 1) // b


def _build():
    nc = bacc.Bacc("TRN2", target_bir_lowering=False, debug=False, num_devices=NCORES)
    AF = mybir.ActivationFunctionType

    # ---- DRAM parameters ----
    din = {}
    for i, sc in enumerate(SCALES):
        din[f't{i}s'] = nc.dram_tensor(f't{i}s', [TD, sc['Nk']], MDT, kind="ExternalInput")
        din[f'v{i}h'] = nc.dram_tensor(f'v{i}h', [sc['cin'], sc['Nq']], MDT, kind="ExternalInput")
        din[f'vp_wT{i}'] = nc.dram_tensor(f'vp_wT{i}', [sc['cin'], TD], MDT, kind="ExternalInput")
        din[f'op_wT{i}'] = nc.dram_tensor(f'op_wT{i}', [TD, sc['cin']], MDT, kind="ExternalInput")
        din[f'vp_b{i}'] = nc.dram_tensor(f'vp_b{i}', [TD], F32, kind="ExternalInput")
        din[f'op_b{i}'] = nc.dram_tensor(f'op_b{i}', [sc['cin']], F32, kind="ExternalInput")
    for nm in ['q', 'k', 'v']:
        din[f'{nm}_wT'] = nc.dram_tensor(f'{nm}_wT', [TD, TD], MDT, kind="ExternalInput")
        din[f'{nm}_b'] = nc.dram_tensor(f'{nm}_b', [TD], F32, kind="ExternalInput")
    din['sg_wT'] = nc.dram_tensor('sg_wT', [4 * TD, TD], MDT, kind="ExternalInput")
    din['sg_b'] = nc.dram_tensor('sg_b', [TD], F32, kind="ExternalInput")
    din['bn_g'] = nc.dram_tensor('bn_g', [TD], F32, kind="ExternalInput")
    din['bn_b'] = nc.dram_tensor('bn_b', [TD], F32, kind="ExternalInput")
    din['fc1_wT'] = nc.dram_tensor('fc1_wT', [TD, 1024], F32, kind="ExternalInput")
    din['fc1_b'] = nc.dram_tensor('fc1_b', [1024], F32, kind="ExternalInput")
    din['fc2h_wT'] = nc.dram_tensor('fc2h_wT', [1024, 1024], F32, kind="ExternalInput")
    din['fc2h_b'] = nc.dram_tensor('fc2h_b', [1024], F32, kind="ExternalInput")

    dout = {}
    for i, sc in enumerate(SCALES):
        dout[f'e{i}'] = nc.dram_tensor(f'e{i}', [sc['cin'], sc['Nq']], F32, kind="ExternalOutput")
    dout['sem'] = nc.dram_tensor('sem', [8, 128], F32, kind="ExternalOutput")

    with tile.TileContext(nc) as tc:
        import contextlib
        ctx = contextlib.ExitStack()
        with ctx:
            bigA = ctx.enter_context(tc.tile_pool(name="bigA", bufs=1))
            persist = ctx.enter_context(tc.tile_pool(name="persist", bufs=1))
            wtB = ctx.enter_context(tc.tile_pool(name="wtB", bufs=2))
            wtA = ctx.enter_context(tc.tile_pool(name="wtA", bufs=1))
            vpP = ctx.enter_context(tc.tile_pool(name="vpP", bufs=2))
            qP = ctx.enter_context(tc.tile_pool(name="qP", bufs=2))
            exP = ctx.enter_context(tc.tile_pool(name="exP", bufs=2))
            avTP = ctx.enter_context(tc.tile_pool(name="avTP", bufs=2))
            avP = ctx.enter_context(tc.tile_pool(name="avP", bufs=2))
            eP = ctx.enter_context(tc.tile_pool(name="eP", bufs=2))
            s123 = ctx.enter_context(tc.tile_pool(name="s123", bufs=1))
            misc = ctx.enter_context(tc.tile_pool(name="misc", bufs=4))
            psA = ctx.enter_context(tc.tile_pool(name="psA", bufs=2, space="PSUM"))
            psB = ctx.enter_context(tc.tile_pool(name="psB", bufs=4, space="PSUM"))
            psC = ctx.enter_context(tc.tile_pool(name="psC", bufs=2, space="PSUM"))
            dramP = ctx.enter_context(tc.tile_pool(name="dramP", bufs=1, space="DRAM"))

            def load_wT(name, K, M):
                """Weight W^T [K, M] -> SBUF [p, nk, M] tile, p=min(128,K)."""
                p = min(128, K)
                nk = _ceil(K, 128)
                t = persist.tile([p, nk, M], MDT, tag=name)
                nc.sync.dma_start(out=t[:], in_=din[name].ap().rearrange(
                    "(k p) m -> p k m", p=p))
                return t

            def load_wT_pool(pool, name, K, M):
                p = min(128, K)
                nk = _ceil(K, 128)
                t = pool.tile([p, nk, M], MDT, tag="wslot")
                nc.sync.dma_start(out=t[:], in_=din[name].ap().rearrange(
                    "(k p) m -> p k m", p=p))
                return t

            def load_bias(name, C):
                p = min(128, C)
                nj = _ceil(C, 128)
                t = persist.tile([p, nj], F32, tag=name)
                nc.sync.dma_start(out=t[:], in_=din[name].ap().rearrange(
                    "(j p) -> p j", p=p))
                return t

            # ---- persistent weights ----
            q_wT = load_wT('q_wT', TD, TD)
            k_wT = load_wT('k_wT', TD, TD)
            v_wT = load_wT('v_wT', TD, TD)
            q_b = load_bias('q_b', TD)
            k_b = load_bias('k_b', TD)
            vp_bs = [load_bias(f'vp_b{i}', TD) for i in range(4)]
            op_bs = [load_bias(f'op_b{i}', sc['cin']) for i, sc in enumerate(SCALES)]
            sg_b = load_bias('sg_b', TD)
            bn_g = load_bias('bn_g', TD)
            bn_b = load_bias('bn_b', TD)
            fc1_b = load_bias('fc1_b', 1024)
            fc2h_b = load_bias('fc2h_b', 1024)

            # v_b broadcast across partitions [128, 256] (free dim = out channel)
            vb_bc = persist.tile([128, TD], F32, tag="vb_bc")
            nc.gpsimd.dma_start(out=vb_bc[:], in_=din['v_b'].ap().unsqueeze(0).broadcast_to([128, TD]))

            ident = persist.tile([128, 128], F32, tag="ident")
            make_identity(nc, ident[:])

            ones_t = persist.tile([128, 128], F32, tag="ones")
            nc.vector.memset(ones_t[:], 1.0)

            # ---- t maps ----
            t_sbs = []
            for i, sc in enumerate(SCALES):
                pool = bigA if i == 0 else persist
                t_t = pool.tile([128, 2, sc['Nk']], MDT, tag="big" if i == 0 else f"t{i}")
                nc.sync.dma_start(out=t_t[:], in_=din[f't{i}s'].ap().rearrange(
                    "(j p) n -> p j n", p=128))
                t_sbs.append(t_t)

            v_sbs = []
            for i, sc in enumerate(SCALES):
                cin = sc['cin']
                p = min(128, cin)
                nci = _ceil(cin, 128)
                v_t = persist.tile([p, nci, sc['Nq']], MDT, tag=f"v{i}")
                nc.sync.dma_start(out=v_t[:], in_=din[f'v{i}h'].ap().rearrange(
                    "(k p) n -> p k n", p=p))
                v_sbs.append(v_t)

            # ---- pooling -> cat chunks (8 x [128, 64]) ----
            cat_sb = persist.tile([128, 6, 64], MDT, tag="cat")
            for i, sc in enumerate(SCALES[:3]):
                yg, yd = sc['yg'], sc['yd']
                for j in range(2):
                    src = t_sbs[i][:, j, :].rearrange(
                        "p (yg yd xg xd) -> p yg xg yd xd", yg=yg, yd=yd, xg=yg, xd=yd)
                    dst = cat_sb[:, i * 2 + j, :].rearrange("p (a b) -> p a b", a=8)
                    with nc.allow_low_precision(reason="fp32r out, fp32 internal accum"):
                        nc.vector.reduce_sum(out=dst, in_=src, axis=mybir.AxisListType.XY)
            cat_chunks = [cat_sb[:, m, :] for m in range(6)] + \
                         [t_sbs[3][:, 0, :], t_sbs[3][:, 1, :]]

            # ---- sg conv + batch stats ----
            sg_wT = load_wT_pool(wtA, 'sg_wT', 4 * TD, TD)
            x_sb = persist.tile([128, 2, 64], F32, tag="x_sb")
            stats = persist.tile([128, 2, 2], F32, tag="stats")
            sq_t = misc.tile([128, 64], F32, tag="sq")
            for j in range(2):
                ps = psC.tile([128, 64], F32, tag="psC")
                for kc in range(8):
                    nc.tensor.matmul(ps[:], sg_wT[:, kc, j * 128:(j + 1) * 128],
                                     cat_chunks[kc], start=(kc == 0), stop=(kc == 7))
                nc.vector.tensor_scalar_add(out=x_sb[:, j, :], in0=ps[:], scalar1=sg_b[:, j:j + 1])
                nc.vector.reduce_sum(out=stats[:, j, 0:1], in_=x_sb[:, j, :],
                                     axis=mybir.AxisListType.X)
                nc.scalar.activation(out=sq_t[:], in_=x_sb[:, j, :], func=AF.Square,
                                     accum_out=stats[:, j, 1:2])

            cc_in = dramP.tile([TD, 2], F32)
            cc_out = dramP.tile([TD, 2], F32)
            nc.sync.dma_start(out=cc_in[:].rearrange("(j p) s -> p j s", p=128), in_=stats[:])
            nc.gpsimd.collective_compute(
                "AllReduce", mybir.AluOpType.add,
                replica_groups=[list(range(NCORES))],
                ins=[cc_in.opt()], outs=[cc_out.opt()],
            )
            st_sb = persist.tile([128, 2, 2], F32, tag="st_sb")
            nc.sync.dma_start(out=st_sb[:], in_=cc_out[:].rearrange("(j p) s -> p j s", p=128))

            # ---- generic attention ----
            def attention(i, k_dst, vT_dst):
                sc = SCALES[i]
                cin, Nk, Nq = sc['cin'], sc['Nk'], sc['Nq']
                nm = _ceil(Nk, 128)
                mP = min(128, Nk)
                nci = _ceil(cin, 128)
                cinP = min(128, cin)
                wn = min(512, Nq)
                nstrips = Nq // wn
                nsub = _ceil(wn, 128)
                nP = min(128, wn)
                t_sb = t_sbs[i]
                v_sb = v_sbs[i]
                vp_wT = load_wT_pool(wtB, f'vp_wT{i}', cin, TD)
                op_wT = load_wT_pool(wtB, f'op_wT{i}', TD, cin)

                # K [256, Nk]
                for j in range(2):
                    for s in range(_ceil(Nk, 512)):
                        w = min(512, Nk - s * 512)
                        ps = psA.tile([128, w], F32, tag="psA")
                        for kc in range(2):
                            nc.tensor.matmul(ps[:], k_wT[:, kc, j * 128:(j + 1) * 128],
                                             t_sb[:, kc, s * 512:s * 512 + w],
                                             start=(kc == 0), stop=(kc == 1))
                        nc.vector.tensor_scalar_add(out=k_dst[:, j, s * 512:s * 512 + w],
                                                    in0=ps[:], scalar1=k_b[:, j:j + 1])

                # V^T [Nk, 260] with ones columns at 256:260
                nc.vector.tensor_copy(
                    vT_dst[:mP, :, TD:TD + 4],
                    ones_t[:mP, :nm * 4].rearrange("p (a b) -> p a b", b=4))
                for m in range(nm):
                    ps = psB.tile([mP, TD], F32, tag="psBv")
                    for kc in range(2):
                        nc.tensor.matmul(ps[:], t_sb[:, kc, m * 128:m * 128 + mP],
                                         v_wT[:, kc, :], start=(kc == 0), stop=(kc == 1))
                    nc.vector.tensor_add(vT_dst[:mP, m, 0:TD], ps[:], vb_bc[:mP, :])

                for s in range(nstrips):
                    sl = slice(s * wn, (s + 1) * wn)
                    # vp strip [256, wn]
                    vp_t = vpP.tile([128, 2, wn], MDT, tag="vp")
                    for j in range(2):
                        ps = psA.tile([128, wn], F32, tag="psA")
                        for kc in range(nci):
                            kP = min(128, cin - kc * 128)
                            nc.tensor.matmul(ps[:], vp_wT[:kP, kc, j * 128:(j + 1) * 128],
                                             v_sb[:kP, kc, sl],
                                             start=(kc == 0), stop=(kc == nci - 1))
                        nc.vector.tensor_scalar_add(out=vp_t[:, j, :], in0=ps[:],
                                                    scalar1=vp_bs[i][:, j:j + 1])
                    # q strip
                    q_t = qP.tile([128, 2, wn], MDT, tag="q")
                    for j in range(2):
                        ps = psA.tile([128, wn], F32, tag="psA")
                        for kc in range(2):
                            nc.tensor.matmul(ps[:], q_wT[:, kc, j * 128:(j + 1) * 128],
                                             vp_t[:, kc, :], start=(kc == 0), stop=(kc == 1))
                        nc.vector.tensor_scalar_add(out=q_t[:, j, :], in0=ps[:],
                                                    scalar1=q_b[:, j:j + 1])
                    # attention
                    avps = [psB.tile([nP, TD + 4], F32, tag="psBv", name=f"avp{i}_{s}_{x}") for x in range(nsub)]
                    for m in range(nm):
                        sps = psA.tile([mP, wn], F32, tag="psA")
                        for kc in range(2):
                            nc.tensor.matmul(sps[:], k_dst[:, kc, m * 128:m * 128 + mP],
                                             q_t[:, kc, :], start=(kc == 0), stop=(kc == 1))
                        ex = exP.tile([mP, wn], MDT, tag="ex")
                        nc.scalar.activation(out=ex[:], in_=sps[:], func=AF.Exp,
                                             scale=float(1.0 / np.sqrt(TD)))
                        for ns in range(nsub):
                            nw = min(128, wn - ns * 128)
                            nc.tensor.matmul(avps[ns][:, :], ex[:, ns * 128:ns * 128 + nw],
                                             vT_dst[:mP, m, :],
                                             start=(m == 0), stop=(m == nm - 1),
                                             skip_group_check=True)
                    e_t = eP.tile([cinP, nci, wn], F32, tag="e")
                    for ns in range(nsub):
                        nw = min(128, wn - ns * 128)
                        rec = misc.tile([nP, 1], F32, tag="rec")
                        nc.vector.reciprocal(rec[:nw], avps[ns][:nw, TD:TD + 1])
                        avT_t = avTP.tile([nP, TD], F32, tag="avT")
                        nc.scalar.activation(out=avT_t[:nw], in_=avps[ns][:nw, 0:TD],
                                             func=AF.Copy, scale=rec[:nw])
                        av_t = avP.tile([128, 2, nP], MDT, tag="av")
                        for jc in range(2):
                            tp = psC.tile([128, nP], F32, tag="psC")
                            nc.tensor.transpose(tp[:, :nw], avT_t[:nw, jc * 128:(jc + 1) * 128],
                                                ident[:nw, :nw])
                            nc.vector.tensor_copy(av_t[:, jc, :nw], tp[:, :nw])
                        for oc in range(nci):
                            oP = min(128, cin - oc * 128)
                            po = psC.tile([oP, nP], F32, tag="psC")
                            for kc in range(2):
                                nc.tensor.matmul(po[:, :nw], op_wT[:, kc, oc * 128:oc * 128 + oP],
                                                 av_t[:, kc, :nw], start=(kc == 0), stop=(kc == 1))
                            nsl = slice(ns * 128, ns * 128 + nw)
                            nc.vector.tensor_scalar_add(out=e_t[:oP, oc, nsl], in0=po[:, :nw],
                                                        scalar1=op_bs[i][:oP, oc:oc + 1])
                            nc.vector.tensor_add(e_t[:oP, oc, nsl], e_t[:oP, oc, nsl],
                                                 v_sb[:oP, oc, s * wn + ns * 128:s * wn + ns * 128 + nw])
                    nc.sync.dma_start(
                        out=dout[f'e{i}'].ap().rearrange("(k p) n -> p k n", p=cinP)[:, :, sl],
                        in_=e_t[:])

            # scale 0 with dedicated big tiles
            k0 = persist.tile([128, 2, SCALES[0]['Nk']], MDT, tag="k0")
            vT0 = persist.tile([128, 32, TD + 4], MDT, tag="vT0")
            attention(0, k0, vT0)

            # fc2 weights reuse the t0 slot (t0s dead after K/V^T/pooling)
            fc2h_wT = bigA.tile([128, 8, 1024], F32, tag="big")
            nc.sync.dma_start(out=fc2h_wT[:], in_=din['fc2h_wT'].ap().rearrange(
                "(k p) m -> p k m", p=128))

            # scales 1-3 share one set of slots
            k123 = [s123.tile([128, 2, SCALES[ii]["Nk"]], MDT, tag="k123", name=f"k123_{ii}") for ii in (1, 2, 3)]
            vT123 = [s123.tile([min(128, SCALES[ii]["Nk"]), _ceil(SCALES[ii]["Nk"], 128), TD + 4],
                               MDT, tag="vT123", name=f"vT123_{ii}") for ii in (1, 2, 3)]
            for ii in (1, 2, 3):
                attention(ii, k123[ii - 1], vT123[ii - 1])

            # ---- BN apply + fc path ----
            r_sb = persist.tile([128, 2], F32, tag="r_sb")
            xr_t = misc.tile([128, 64], F32, tag="xr")
            eps_t = persist.tile([128, 1], F32, tag="eps")
            nc.vector.memset(eps_t[:], 1e-5)
            for j in range(2):
                mu = misc.tile([128, 1], F32, tag="bn1")
                ex2 = misc.tile([128, 1], F32, tag="bn2")
                nc.scalar.mul(mu[:], st_sb[:, j, 0:1], 1.0 / 512.0)
                nc.scalar.mul(ex2[:], st_sb[:, j, 1:2], 1.0 / 512.0)
                mu2 = misc.tile([128, 1], F32, tag="bn3")
                nc.vector.tensor_mul(mu2[:], mu[:], mu[:])
                var = misc.tile([128, 1], F32, tag="bn4")
                nc.vector.tensor_sub(var[:], ex2[:], mu2[:])
                sd = misc.tile([128, 1], F32, tag="bn5")
                nc.scalar.activation(out=sd[:], in_=var[:], func=AF.Sqrt, bias=eps_t[:])
                rstd = misc.tile([128, 1], F32, tag="bn6")
                nc.vector.reciprocal(rstd[:], sd[:])
                a_t = misc.tile([128, 1], F32, tag="bn7")
                nc.vector.tensor_mul(a_t[:], rstd[:], bn_g[:, j:j + 1])
                amu = misc.tile([128, 1], F32, tag="bn8")
                nc.vector.tensor_mul(amu[:], a_t[:], mu[:])
                bb = misc.tile([128, 1], F32, tag="bn9")
                nc.vector.tensor_sub(bb[:], bn_b[:, j:j + 1], amu[:])
                nc.scalar.activation(out=xr_t[:], in_=x_sb[:, j, :], func=AF.Relu,
                                     scale=a_t[:], bias=bb[:])
                with nc.allow_low_precision(reason="fp32r out, fp32 internal accum"):
                    nc.vector.reduce_sum(out=r_sb[:, j:j + 1], in_=xr_t[:],
                                         axis=mybir.AxisListType.X)

            fc1_wT = wtA.tile([128, 2, 1024], F32, tag="wslot")
            nc.sync.dma_start(out=fc1_wT[:], in_=din['fc1_wT'].ap().rearrange(
                "(k p) m -> p k m", p=128))
            h_sb = persist.tile([128, 8], F32, tag="h_sb")
            for hc in range(8):
                ps = psC.tile([128, 1], F32, tag="psC")
                for kc in range(2):
                    nc.tensor.matmul(ps[:], fc1_wT[:, kc, hc * 128:(hc + 1) * 128],
                                     r_sb[:, kc:kc + 1], start=(kc == 0), stop=(kc == 1))
                nc.scalar.activation(out=h_sb[:, hc:hc + 1], in_=ps[:], func=AF.Relu,
                                     bias=fc1_b[:, hc:hc + 1])
            sem_sb = persist.tile([128, 8], F32, tag="sem_sb")
            for oc in range(8):
                ps = psC.tile([128, 1], F32, tag="psC")
                for kc in range(8):
                    nc.tensor.matmul(ps[:], fc2h_wT[:, kc, oc * 128:(oc + 1) * 128],
                                     h_sb[:, kc:kc + 1], start=(kc == 0), stop=(kc == 7))
                nc.vector.tensor_scalar_add(out=sem_sb[:, oc:oc + 1], in0=ps[:],
                                            scalar1=fc2h_b[:, oc:oc + 1])
            nc.sync.dma_start(out=dout['sem'].ap().rearrange("o p -> p o"), in_=sem_sb[:])

    nc.compile()
    return nc


_NC = None


def _get_nc():
    global _NC
    if _NC is None:
        _NC = _build()
    return _NC


def _prep_maps(inputs):
    f = np.float32
    ins = {k: np.asarray(v, dtype=f) for k, v in inputs.items()}
    pool_div = [64.0, 16.0, 4.0, 1.0]
    sg_wT = np.ascontiguousarray(ins['sg_w'].T).astype(f)  # [1024, 256]
    for i in range(4):
        sg_wT[i * TD:(i + 1) * TD, :] /= f(pool_div[i])
    fc1_wT = np.ascontiguousarray((ins['fc1_w'] / 64.0).T).astype(f)  # [256, 1024]
    shared = {
        'q_wT': np.ascontiguousarray(ins['q_w'].T), 'q_b': ins['q_b'],
        'k_wT': np.ascontiguousarray(ins['k_w'].T), 'k_b': ins['k_b'],
        'v_wT': np.ascontiguousarray(ins['v_w'].T), 'v_b': ins['v_b'],
        'sg_wT': sg_wT, 'sg_b': ins['sg_b'],
        'bn_g': ins['bn_g'], 'bn_b': ins['bn_b'],
        'fc1_wT': fc1_wT, 'fc1_b': ins['fc1_b'],
    }
    for i in range(4):
        shared[f'vp_wT{i}'] = np.ascontiguousarray(ins[f'vp_w{i}'].T)
        shared[f'vp_b{i}'] = ins[f'vp_b{i}']
        shared[f'op_wT{i}'] = np.ascontiguousarray(ins[f'op_w{i}'].T)
        shared[f'op_b{i}'] = ins[f'op_b{i}']
    maps = []
    for c in range(NCORES):
        b, h = c // 2, c % 2
        m = dict(shared)
        for i, sc in enumerate(SCALES):
            cin, Nk, Nq = sc['cin'], sc['Nk'], sc['Nq']
            m[f't{i}s'] = np.ascontiguousarray(ins[f't{i}'][b].reshape(TD, Nk))
            vfull = ins[f'v{i}'][b].reshape(cin, Nk)
            if i == 3:
                m[f'v{i}h'] = np.ascontiguousarray(vfull)
            else:
                m[f'v{i}h'] = np.ascontiguousarray(vfull[:, h * Nq:(h + 1) * Nq])
        m['fc2h_wT'] = np.ascontiguousarray(ins['fc2_w'][h * 1024:(h + 1) * 1024, :].T)
        m['fc2h_b'] = np.ascontiguousarray(ins['fc2_b'][h * 1024:(h + 1) * 1024])
        maps.append(m)
    return maps


def kernel(**inputs):
    nc = _get_nc()
    maps = _prep_maps(inputs)
    trace = bool(int(os.environ.get("BASS_TRACE", "0")))
    res = run_bass_kernel_spmd(nc, maps, list(range(NCORES)), trace=trace)
    LAST['res'] = res
    B = 4
    ench = []
    for i, sc in enumerate(SCALES):
        cin, Nk = sc['cin'], sc['Nk']
        HW = int(np.sqrt(Nk))
        out = np.zeros((B, cin, Nk), np.float32)
        for b in range(B):
            if i == 3:
                out[b] = res.results[2 * b][f'e{i}']
            else:
                out[b, :, :Nk // 2] = res.results[2 * b][f'e{i}']
                out[b, :, Nk // 2:] = res.results[2 * b + 1][f'e{i}']
        ench.append(out.reshape(B, cin, HW, HW))
    sem = np.zeros((B, 2048), np.float32)
    for b in range(B):
        sem[b, :1024] = res.results[2 * b]['sem'].reshape(1024)
        sem[b, 1024:] = res.results[2 * b + 1]['sem'].reshape(1024)
    semantic = sem.reshape(B, 2, 1024)
    return (semantic, ench[0], ench[1], ench[2], ench[3])
